# revision 41
# baseline (speedup 1.0000x reference)
"""Trainium2 Bass kernel for a per-head dense MLP (CriticCVaR head).

Computes, per head t:
    h   = silu(states[t] @ W1[t] + b1[t])        # [B, S] @ [S, H]
    out = (h @ W2[t] + b2[t]).squeeze(-1)        # [B, H] @ [H, 1] -> [B]

Sharding: heads T=32 split across 8 NeuronCores (4 heads/core, full batch).

Device layout / schedule (measured ~60us vs the 90us fp16 baseline):
  - states are pre-transposed on the host to [S, B] and shipped as
    fp8e3 (e3m4): the PE accepts a mixed-dtype matmul (fp16 stationary
    W1 x fp8e3 moving X) at full rate, so X DMA traffic halves while
    the W1 operand keeps fp16 precision (measured end-to-end rel err
    ~1.4e-2 vs the 2e-2 budget).
  - the batch is processed in column blocks of BW; per block the PE
    stream is MM1(blk, t0..t3) then MM2(blk-1): the second matmul runs
    one block behind so its silu(z) dependency is always satisfied and
    the PE never idles waiting on the activation engine.
  - the four heads' M=32 second matmuls are col-tiled (tile_position)
    onto partitions 0/32/64/96; emitted hh-major and back-to-back they
    overlap almost entirely in the PE array (the 3 trailing quadrant
    matmuls retire in a few ns), so MM2 costs ~1/4 of its serial time.
  - PSUM: p1 is TRIPLE-buffered (3 x 2 banks) so MM1 of head t+2 never
    waits on silu of head t; MM2 uses single-bank p2 tiles (2 x 1 bank).
    This p1 slack was worth ~4us.
  - z for the 4 heads is packed in one SBUF tile per block so the MM2
    quadrant group shares one dependency source and stays contiguous
    in the schedule.
  - X rides the sync HWDGE ring in consumption order (512-col chunks
    for the first tile, 1024 for blocks 0-1, 2048 after -- the ~650ns
    per-trigger sequencer cost must stay ahead of the PE); consts ride
    the scalar ring, whose queue must otherwise stay clear (ACT has
    exec-queue depth 0, so a DMA trigger behind a silu stalls it);
    output stores also ride sync (the GPSIMD SWDGE path added ~1us of
    store latency at the drain).
"""

from contextlib import ExitStack

import numpy as np

T, B, S, H = 32, 8192, 256, 128
NCORES = 8
TLOC = T // NCORES          # heads per core
KCH = S // 128              # contraction chunks (S on partitions)
MMN = 512                   # matmul free dim (one PSUM bank of fp32)
BW = 1024                   # batch columns per pipeline block


def build_nc(b_total: int = B, bw: int = BW, use_silu: bool = True):
    import concourse.mybir as mybir
    import concourse.tile as tile
    from concourse import bacc

    fp16 = mybir.dt.float16
    fp32 = mybir.dt.float32
    f83 = mybir.dt.float8e3
    # last two blocks are half-width: the final block's serial
    # MM1->silu->MM2->evac->store drain is ~2x shorter
    widths = [bw] * (b_total // bw - 1) + [bw // 2, bw // 2]
    starts = [sum(widths[:i]) for i in range(len(widths))]
    nbb = len(widths)

    nc = bacc.Bacc("TRN2", target_bir_lowering=False, debug=False)
    xT = nc.dram_tensor("xT", [TLOC, KCH, 128, b_total], f83, kind="ExternalInput")
    w1 = nc.dram_tensor("w1", [128, TLOC * KCH * H], fp16, kind="ExternalInput")
    b1 = nc.dram_tensor("b1", [H, TLOC], fp32, kind="ExternalInput")
    w2 = nc.dram_tensor("w2", [H, 32 * TLOC], fp16, kind="ExternalInput")
    b2 = nc.dram_tensor("b2", [128, 1], fp32, kind="ExternalInput")  # b2[t] at row 32t
    out = nc.dram_tensor("out", [TLOC, b_total], fp32, kind="ExternalOutput")

    silu = mybir.ActivationFunctionType.Silu

    with ExitStack() as ctx:
        tc = ctx.enter_context(tile.TileContext(nc))
        cpool = ctx.enter_context(tc.tile_pool(name="const", bufs=1))
        xpool = ctx.enter_context(tc.tile_pool(name="x", bufs=1))
        zpool = ctx.enter_context(tc.tile_pool(name="z", bufs=2))
        spool = ctx.enter_context(tc.tile_pool(name="s", bufs=2))
        opool = ctx.enter_context(tc.tile_pool(name="o", bufs=6))
        p1pool = ctx.enter_context(tc.tile_pool(name="p1", bufs=3, space="PSUM"))
        p2pool = ctx.enter_context(tc.tile_pool(name="p2", bufs=2, space="PSUM"))

        # Consts ride the scalar ring (issued before any silu queues up) so
        # the sync ring starts streaming X immediately. w1 is split so the
        # first matmul only waits on head 0's 64KB slice.
        w1sb = cpool.tile([128, TLOC * KCH * H], fp16)
        nc.scalar.dma_start(w1sb[:, 0:H], w1.ap()[:, 0:H])
        nc.scalar.dma_start(w1sb[:, H : KCH * H], w1.ap()[:, H : KCH * H])
        b1sb = cpool.tile([H, TLOC], fp32)
        nc.scalar.dma_start(b1sb[:, :], b1.ap()[:, :])
        w2sb = cpool.tile([H, 32 * TLOC], fp16)
        nc.scalar.dma_start(w2sb[:, :], w2.ap()[:, :])
        b2sb = cpool.tile([128, 1], fp32)
        nc.scalar.dma_start(b2sb[:, :], b2.ap()[:, :])
        nc.scalar.dma_start(w1sb[:, KCH * H :], w1.ap()[:, KCH * H :])

        # Warm-up ops: absorb the const-DMA waits and pre-load the Silu
        # activation table before the steady-state loop.
        warm_a = cpool.tile([H, TLOC], fp32)
        nc.scalar.activation(
            warm_a[:, :],
            b1sb[:, :],
            silu if use_silu else mybir.ActivationFunctionType.Sigmoid,
        )
        warm_v = cpool.tile([128, 1], fp32)
        nc.vector.tensor_scalar_add(warm_v[:, :], b1sb[:, 0:1], 0.0)

        # PE warm-up: a few discarded matmuls on head 0's weights fill the
        # PE's DMA-wait window at startup and ramp the clock out of the low
        # p-state before the first real MM1.
        for i in range(2):
            pw = p2pool.tile([128, MMN], fp32, tag="p2", name="pw")
            nc.tensor.matmul(
                pw[:, :],
                w1sb[:, 0:128],
                w1sb[:, 0:MMN],
                start=True,
                stop=True,
            )

        # Whole-core X resident in SBUF: one persistent tile per (t, k),
        # filled by per-block column-chunk DMAs in consumption order so
        # early matmuls only wait on their own chunk. All X rides the sync
        # HWDGE ring (the scalar queue must stay clear: ACT has exec-queue
        # depth 0, so a DMA trigger queued behind a silu stalls the ring).
        xtiles = {}
        for t in range(TLOC):
            for k in range(KCH):
                xtiles[t, k] = xpool.tile(
                    [128, b_total], f83, tag=f"x{t}{k}", name=f"xt{t}{k}"
                )

        def x_chunks():
            # (t, k, lo, sz) in consumption order; head 0 of block 0 is
            # sub-chunked so the first matmuls wait on partial transfers
            # only, and blocks >= 2 use double-width chunks so the sync
            # sequencer's ~650ns/trigger issue rate stays well ahead of
            # the PE's ~6us/block consumption rate.
            for bb in range(2):
                c0 = bb * bw
                for t in range(TLOC):
                    for k in range(KCH):
                        sz = 512 if bb == 0 and t == 0 else bw
                        for lo in range(c0, c0 + bw, sz):
                            yield t, k, lo, sz
            for c0 in range(2 * bw, b_total, 2 * bw):
                for t in range(TLOC):
                    for k in range(KCH):
                        yield t, k, c0, 2 * bw

        for t, k, lo, sz in x_chunks():
            nc.sync.dma_start(
                xtiles[t, k][:, lo : lo + sz],
                xT.ap()[t, k, :, lo : lo + sz],
            )

        def mm1_block(bb):
            c0, w = starts[bb], widths[bb]
            for t in range(TLOC):
                p1 = p1pool.tile([128, bw], fp32, tag="p1")
                for k in range(KCH):
                    for hh in range(w // MMN):
                        hc = hh * MMN
                        nc.tensor.matmul(
                            p1[:, hc : hc + MMN],
                            w1sb[:, (t * KCH + k) * H : (t * KCH + k + 1) * H],
                            xtiles[t, k][:, c0 + hc : c0 + hc + MMN],
                            start=(k == 0),
                            stop=(k == KCH - 1),
                        )
                if t == 0:
                    zs["all"] = zpool.tile(
                        [128, TLOC * bw], fp16, tag="z", name="zall"
                    )
                z = zs["all"][:, t * bw : t * bw + w]
                if use_silu:
                    nc.scalar.activation(
                        z[:, :], p1[:, :w], silu, bias=b1sb[:, t : t + 1]
                    )
                else:
                    # CoreSim fallback: silu(y) = y * sigmoid(y)
                    sg = spool.tile([128, bw], fp16, tag="sg")
                    nc.scalar.activation(
                        sg[:, :w],
                        p1[:, :w],
                        mybir.ActivationFunctionType.Sigmoid,
                        bias=b1sb[:, t : t + 1],
                    )
                    yb = spool.tile([128, bw], fp32, tag="yb")
                    nc.vector.tensor_scalar_add(
                        yb[:, :w], p1[:, :w], b1sb[:, t : t + 1]
                    )
                    nc.vector.tensor_mul(z[:, :], yb[:, :w], sg[:, :w])


        def mm2_block(bb, zprev):
            c0, w = starts[bb], widths[bb]
            # hh-major quadrant groups: 4 back-to-back matmuls on distinct
            # PE column-tiles overlap almost entirely in the array; p2 is a
            # single PSUM bank per group so p1 can triple-buffer.
            for hh in range(w // MMN):
                hc = hh * MMN
                p2 = p2pool.tile([128, MMN], fp32, tag="p2", name="p2")
                for t in range(TLOC):
                    # M=32 with w2[t] replicated across columns: all rows of
                    # the col-group get the head's result (same N-cycle cost
                    # as M=1) so the PSUM tile is fully initialized.
                    nc.tensor.matmul(
                        p2[32 * t : 32 * t + 32, :],
                        w2sb[:, 32 * t : 32 * t + 32],
                        zprev[:, t * bw + hc : t * bw + hc + MMN],
                        start=True,
                        stop=True,
                        tile_position=(0, 32 * t),
                    )
                o = opool.tile([128, MMN], fp32, tag="o", name="o")
                nc.vector.tensor_scalar_add(o[:, :], p2[:, :], b2sb[:, 0:1])
                nc.sync.dma_start(
                    out.ap()[:, c0 + hc : c0 + hc + MMN],
                    o[0:97:32, :],
                )

        # MM2 lags MM1 by one block so its silu dependencies are always met.
        zs = {}
        zprev = None
        for bb in range(nbb):
            mm1_block(bb)
            if zprev is not None:
                mm2_block(bb - 1, zprev)
            zprev = zs["all"]
        mm2_block(nbb - 1, zprev)

    nc.compile()
    return nc


def make_in_maps(states_batch, W1, b1, W2, b2):
    import ml_dtypes

    states_batch = np.asarray(states_batch)
    W1, b1, W2, b2 = (np.asarray(a) for a in (W1, b1, W2, b2))
    b_total = states_batch.shape[1]
    in_maps = []
    for c in range(NCORES):
        sl = slice(c * TLOC, (c + 1) * TLOC)
        xT = (
            states_batch[sl]
            .transpose(0, 2, 1)
            .astype(ml_dtypes.float8_e3m4)
            .reshape(TLOC, KCH, 128, b_total)
        )
        w1h = (
            W1[sl]
            .reshape(TLOC, KCH, 128, H)
            .transpose(2, 0, 1, 3)
            .reshape(128, TLOC * KCH * H)
            .astype(np.float16)
        )
        b1h = np.ascontiguousarray(b1[sl].T).astype(np.float32)
        w2h = np.repeat(
            np.ascontiguousarray(W2[sl][:, :, 0].T).astype(np.float16), 32, axis=1
        )
        b2h = np.repeat(b2[sl, 0].astype(np.float32), 32).reshape(128, 1)
        in_maps.append({"xT": xT, "w1": w1h, "b1": b1h, "w2": w2h, "b2": b2h})
    return in_maps


def run(inputs: dict, trace: bool = False):
    from concourse import bass_utils

    nc = build_nc()
    in_maps = make_in_maps(**inputs)
    res = bass_utils.run_bass_kernel_spmd(
        nc, in_maps, core_ids=list(range(NCORES)), trace=trace
    )
    out = np.concatenate([r["out"] for r in res.results], axis=0)
    return out, res


def kernel(**inputs) -> np.ndarray:
    out, _ = run(inputs)
    return out


# revision 42
# speedup vs baseline: 1.0037x; 1.0037x over previous
"""Trainium2 Bass kernel for a per-head dense MLP (CriticCVaR head).

Computes, per head t:
    h   = silu(states[t] @ W1[t] + b1[t])        # [B, S] @ [S, H]
    out = (h @ W2[t] + b2[t]).squeeze(-1)        # [B, H] @ [H, 1] -> [B]

Sharding: heads T=32 split across 8 NeuronCores (4 heads/core, full batch).

Device layout / schedule (measured ~60us vs the 90us fp16 baseline):
  - states are pre-transposed on the host to [S, B] and shipped as
    fp8e3 (e3m4): the PE accepts a mixed-dtype matmul (fp16 stationary
    W1 x fp8e3 moving X) at full rate, so X DMA traffic halves while
    the W1 operand keeps fp16 precision (measured end-to-end rel err
    ~1.4e-2 vs the 2e-2 budget).
  - the batch is processed in column blocks of BW; per block the PE
    stream is MM1(blk, t0..t3) then MM2(blk-1): the second matmul runs
    one block behind so its silu(z) dependency is always satisfied and
    the PE never idles waiting on the activation engine.
  - the four heads' M=32 second matmuls are col-tiled (tile_position)
    onto partitions 0/32/64/96; emitted hh-major and back-to-back they
    overlap almost entirely in the PE array (the 3 trailing quadrant
    matmuls retire in a few ns), so MM2 costs ~1/4 of its serial time.
  - PSUM: p1 is TRIPLE-buffered (3 x 2 banks) so MM1 of head t+2 never
    waits on silu of head t; MM2 uses single-bank p2 tiles (2 x 1 bank).
    This p1 slack was worth ~4us.
  - z for the 4 heads is packed in one SBUF tile per block so the MM2
    quadrant group shares one dependency source and stays contiguous
    in the schedule.
  - X rides the sync HWDGE ring in consumption order (512-col chunks
    for the first tile, 1024 for blocks 0-1, 2048 after -- the ~650ns
    per-trigger sequencer cost must stay ahead of the PE); consts ride
    the scalar ring, whose queue must otherwise stay clear (ACT has
    exec-queue depth 0, so a DMA trigger behind a silu stalls it);
    output stores also ride sync (the GPSIMD SWDGE path added ~1us of
    store latency at the drain).
"""

from contextlib import ExitStack

import numpy as np

T, B, S, H = 32, 8192, 256, 128
NCORES = 8
TLOC = T // NCORES          # heads per core
KCH = S // 128              # contraction chunks (S on partitions)
MMN = 512                   # matmul free dim (one PSUM bank of fp32)
BW = 1024                   # batch columns per pipeline block


def build_nc(b_total: int = B, bw: int = BW, use_silu: bool = True):
    import concourse.mybir as mybir
    import concourse.tile as tile
    from concourse import bacc

    fp16 = mybir.dt.float16
    fp32 = mybir.dt.float32
    f83 = mybir.dt.float8e3
    # last two blocks are half-width: the final block's serial
    # MM1->silu->MM2->evac->store drain is ~2x shorter
    widths = [bw] * (b_total // bw - 1) + [bw // 2, bw // 2]
    starts = [sum(widths[:i]) for i in range(len(widths))]
    nbb = len(widths)

    nc = bacc.Bacc("TRN2", target_bir_lowering=False, debug=False)
    xT = nc.dram_tensor("xT", [TLOC, KCH, 128, b_total], f83, kind="ExternalInput")
    w1 = nc.dram_tensor("w1", [128, TLOC * KCH * H], fp16, kind="ExternalInput")
    b1 = nc.dram_tensor("b1", [H, TLOC], fp32, kind="ExternalInput")
    w2 = nc.dram_tensor("w2", [H, 32 * TLOC], fp16, kind="ExternalInput")
    b2 = nc.dram_tensor("b2", [128, 1], fp32, kind="ExternalInput")  # b2[t] at row 32t
    out = nc.dram_tensor("out", [TLOC, b_total], fp32, kind="ExternalOutput")

    silu = mybir.ActivationFunctionType.Silu

    with ExitStack() as ctx:
        tc = ctx.enter_context(tile.TileContext(nc))
        cpool = ctx.enter_context(tc.tile_pool(name="const", bufs=1))
        xpool = ctx.enter_context(tc.tile_pool(name="x", bufs=1))
        zpool = ctx.enter_context(tc.tile_pool(name="z", bufs=3))
        spool = ctx.enter_context(tc.tile_pool(name="s", bufs=2))
        opool = ctx.enter_context(tc.tile_pool(name="o", bufs=6))
        p1pool = ctx.enter_context(tc.tile_pool(name="p1", bufs=3, space="PSUM"))
        p2pool = ctx.enter_context(tc.tile_pool(name="p2", bufs=2, space="PSUM"))

        # Consts ride the scalar ring (issued before any silu queues up) so
        # the sync ring starts streaming X immediately. w1 is split so the
        # first matmul only waits on head 0's 64KB slice.
        w1sb = cpool.tile([128, TLOC * KCH * H], fp16)
        nc.scalar.dma_start(w1sb[:, 0:H], w1.ap()[:, 0:H])
        nc.scalar.dma_start(w1sb[:, H : KCH * H], w1.ap()[:, H : KCH * H])
        b1sb = cpool.tile([H, TLOC], fp32)
        nc.scalar.dma_start(b1sb[:, :], b1.ap()[:, :])
        w2sb = cpool.tile([H, 32 * TLOC], fp16)
        nc.scalar.dma_start(w2sb[:, :], w2.ap()[:, :])
        b2sb = cpool.tile([128, 1], fp32)
        nc.scalar.dma_start(b2sb[:, :], b2.ap()[:, :])
        nc.scalar.dma_start(w1sb[:, KCH * H :], w1.ap()[:, KCH * H :])

        # Warm-up ops: absorb the const-DMA waits and pre-load the Silu
        # activation table before the steady-state loop.
        warm_a = cpool.tile([H, TLOC], fp32)
        nc.scalar.activation(
            warm_a[:, :],
            b1sb[:, :],
            silu if use_silu else mybir.ActivationFunctionType.Sigmoid,
        )
        warm_v = cpool.tile([128, 1], fp32)
        nc.vector.tensor_scalar_add(warm_v[:, :], b1sb[:, 0:1], 0.0)

        # PE warm-up: a few discarded matmuls on head 0's weights fill the
        # PE's DMA-wait window at startup and ramp the clock out of the low
        # p-state before the first real MM1.
        for i in range(2):
            pw = p2pool.tile([128, MMN], fp32, tag="p2", name="pw")
            nc.tensor.matmul(
                pw[:, :],
                w1sb[:, 0:128],
                w1sb[:, 0:MMN],
                start=True,
                stop=True,
            )

        # Whole-core X resident in SBUF: one persistent tile per (t, k),
        # filled by per-block column-chunk DMAs in consumption order so
        # early matmuls only wait on their own chunk. All X rides the sync
        # HWDGE ring (the scalar queue must stay clear: ACT has exec-queue
        # depth 0, so a DMA trigger queued behind a silu stalls the ring).
        xtiles = {}
        for t in range(TLOC):
            for k in range(KCH):
                xtiles[t, k] = xpool.tile(
                    [128, b_total], f83, tag=f"x{t}{k}", name=f"xt{t}{k}"
                )

        def x_chunks():
            # (t, k, lo, sz) in consumption order; head 0 of block 0 is
            # sub-chunked so the first matmuls wait on partial transfers
            # only, and blocks >= 2 use double-width chunks so the sync
            # sequencer's ~650ns/trigger issue rate stays well ahead of
            # the PE's ~6us/block consumption rate.
            for bb in range(2):
                c0 = bb * bw
                for t in range(TLOC):
                    for k in range(KCH):
                        sz = 512 if bb == 0 and t == 0 else bw
                        for lo in range(c0, c0 + bw, sz):
                            yield t, k, lo, sz
            for c0 in range(2 * bw, b_total, 2 * bw):
                for t in range(TLOC):
                    for k in range(KCH):
                        yield t, k, c0, 2 * bw

        for t, k, lo, sz in x_chunks():
            nc.sync.dma_start(
                xtiles[t, k][:, lo : lo + sz],
                xT.ap()[t, k, :, lo : lo + sz],
            )

        def mm1_block(bb):
            c0, w = starts[bb], widths[bb]
            for t in range(TLOC):
                p1 = p1pool.tile([128, bw], fp32, tag="p1")
                for k in range(KCH):
                    for hh in range(w // MMN):
                        hc = hh * MMN
                        nc.tensor.matmul(
                            p1[:, hc : hc + MMN],
                            w1sb[:, (t * KCH + k) * H : (t * KCH + k + 1) * H],
                            xtiles[t, k][:, c0 + hc : c0 + hc + MMN],
                            start=(k == 0),
                            stop=(k == KCH - 1),
                        )
                if t == 0:
                    zs["all"] = zpool.tile(
                        [128, TLOC * bw], fp16, tag="z", name="zall"
                    )
                z = zs["all"][:, t * bw : t * bw + w]
                if use_silu:
                    nc.scalar.activation(
                        z[:, :], p1[:, :w], silu, bias=b1sb[:, t : t + 1]
                    )
                else:
                    # CoreSim fallback: silu(y) = y * sigmoid(y)
                    sg = spool.tile([128, bw], fp16, tag="sg")
                    nc.scalar.activation(
                        sg[:, :w],
                        p1[:, :w],
                        mybir.ActivationFunctionType.Sigmoid,
                        bias=b1sb[:, t : t + 1],
                    )
                    yb = spool.tile([128, bw], fp32, tag="yb")
                    nc.vector.tensor_scalar_add(
                        yb[:, :w], p1[:, :w], b1sb[:, t : t + 1]
                    )
                    nc.vector.tensor_mul(z[:, :], yb[:, :w], sg[:, :w])


        def mm2_block(bb, zprev):
            c0, w = starts[bb], widths[bb]
            # hh-major quadrant groups: 4 back-to-back matmuls on distinct
            # PE column-tiles overlap almost entirely in the array; p2 is a
            # single PSUM bank per group so p1 can triple-buffer.
            for hh in range(w // MMN):
                hc = hh * MMN
                p2 = p2pool.tile([128, MMN], fp32, tag="p2", name="p2")
                for t in range(TLOC):
                    # M=32 with w2[t] replicated across columns: all rows of
                    # the col-group get the head's result (same N-cycle cost
                    # as M=1) so the PSUM tile is fully initialized.
                    nc.tensor.matmul(
                        p2[32 * t : 32 * t + 32, :],
                        w2sb[:, 32 * t : 32 * t + 32],
                        zprev[:, t * bw + hc : t * bw + hc + MMN],
                        start=True,
                        stop=True,
                        tile_position=(0, 32 * t),
                    )
                o = opool.tile([128, MMN], fp32, tag="o", name="o")
                nc.vector.tensor_scalar_add(o[:, :], p2[:, :], b2sb[:, 0:1])
                nc.sync.dma_start(
                    out.ap()[:, c0 + hc : c0 + hc + MMN],
                    o[0:97:32, :],
                )

        # MM2 lags MM1 by one block so its silu dependencies are always met.
        zs = {}
        zprev = None
        for bb in range(nbb):
            mm1_block(bb)
            if zprev is not None:
                mm2_block(bb - 1, zprev)
            zprev = zs["all"]
        mm2_block(nbb - 1, zprev)

    nc.compile()
    return nc


def make_in_maps(states_batch, W1, b1, W2, b2):
    import ml_dtypes

    states_batch = np.asarray(states_batch)
    W1, b1, W2, b2 = (np.asarray(a) for a in (W1, b1, W2, b2))
    b_total = states_batch.shape[1]
    in_maps = []
    for c in range(NCORES):
        sl = slice(c * TLOC, (c + 1) * TLOC)
        xT = (
            states_batch[sl]
            .transpose(0, 2, 1)
            .astype(ml_dtypes.float8_e3m4)
            .reshape(TLOC, KCH, 128, b_total)
        )
        w1h = (
            W1[sl]
            .reshape(TLOC, KCH, 128, H)
            .transpose(2, 0, 1, 3)
            .reshape(128, TLOC * KCH * H)
            .astype(np.float16)
        )
        b1h = np.ascontiguousarray(b1[sl].T).astype(np.float32)
        w2h = np.repeat(
            np.ascontiguousarray(W2[sl][:, :, 0].T).astype(np.float16), 32, axis=1
        )
        b2h = np.repeat(b2[sl, 0].astype(np.float32), 32).reshape(128, 1)
        in_maps.append({"xT": xT, "w1": w1h, "b1": b1h, "w2": w2h, "b2": b2h})
    return in_maps


def run(inputs: dict, trace: bool = False):
    from concourse import bass_utils

    nc = build_nc()
    in_maps = make_in_maps(**inputs)
    res = bass_utils.run_bass_kernel_spmd(
        nc, in_maps, core_ids=list(range(NCORES)), trace=trace
    )
    out = np.concatenate([r["out"] for r in res.results], axis=0)
    return out, res


def kernel(**inputs) -> np.ndarray:
    out, _ = run(inputs)
    return out
